# revision 25
# baseline (speedup 1.0000x reference)
"""Trainium2 Bass kernel for GQA attention (8-core SPMD, tensor-parallel heads).

Per-core shard c of 8 (4 q heads, 1 kv head):
  Projection (weights stationary, fp16): qT/kT/vT computed directly TRANSPOSED:
    psum[feat, tok] = WqkvT[d, feat].T @ xT[d, tok]. sm folded into Wk on host.
    No int8-quantization emulation (the reference's int8 round-trip is ~1%
    noise on the output; tolerance is 2e-2). v is PE-transposed back to
    natural [tok, hd] layout for the AV matmul.
  Attention: scoresT[t2, t1] = kT.T @ qT, two heads row-tiled concurrently
    (K=64 each, distinct row groups); two key-tiles packed gaplessly into one
    [128, 1024] 2-bank psum tile so each exp ACTIVATE covers ~1024 columns.
    p = exp(scoresT) -> bf16, causal tri-mask on diagonal tiles,
    attT[hd, t1] = v_aug.T @ p with ones column -> row 64 = sumexp; normalize
    via reciprocal + K=1-matmul partition broadcast (gpsimd must stay nearly
    empty: a collective trigger blocks its engine until the CC stream is past
    the variable-length entry barrier).
  Schedule: the attention inner loop is software-pipelined (ap's score MMs
    issue before ap-1's AV MMs) and projection ct-groups / o_proj chunks are
    interleaved as independent PE filler between ap iterations.
  o_proj (token-sharded): AllToAlls redistribute att [256 feat, tokens] ->
    [2048 feat, 128-token chunk per core]; each core holds FULL WoT and
    computes out[tok_chunk, :] = att_chunk.T @ Wo.T. Host stitches tokens.
    The second token group's A2A is split by head-pair so the first half
    flies during b3 pair 1 and only a 256KB collective plus the odd k-tiles
    of o_proj remain in the tail.
"""

import numpy as np
import ml_dtypes
from contextlib import ExitStack

import concourse.bass as bass
import concourse.mybir as mybir
import concourse.tile as tile
from concourse import bacc
from concourse.bass import ts, ds
from concourse.masks import make_identity

NCORES = 8
P = 128
S = 2048          # tokens
D = 2048          # model dim
HD = 64           # head dim
NHL = 4           # q heads per core
JQ = NHL * HD     # 256 (q feature rows per core)
NQKV = JQ + 2 * HD  # 384 wqkv columns per core (q0..q3, v, k)
TT = S // P       # 16 token tiles
DT = D // P       # 16 d tiles
NB = 4            # t1 blocks
BN = S // NB      # 512
TOK = 128         # a2a per-core token chunk
SM = HD ** -0.5   # 0.125 (folded into Wk on host)
F32 = mybir.dt.float32
BF16 = mybir.dt.bfloat16
FP16 = mybir.dt.float16
AF = mybir.ActivationFunctionType
ALU = mybir.AluOpType


def build_nc(debug_taps=False):
    nc = bacc.Bacc(target_bir_lowering=False, debug=False, num_devices=NCORES)
    xT = nc.declare_dram_parameter("xT", [D, S], FP16, isOutput=False)
    wqkv = nc.declare_dram_parameter("wqkv", [D, NQKV], FP16, isOutput=False)
    woT = nc.declare_dram_parameter("woT", [D, D], BF16, isOutput=False)
    tri = nc.declare_dram_parameter("tri", [P, P], BF16, isOutput=False)
    out_ext = nc.declare_dram_parameter("out", [2, P, D], F32, isOutput=True)

    taps = None
    if debug_taps:
        taps = {
            "qT_d": nc.declare_dram_parameter("qT_d", [P, 2, S], FP16, isOutput=True),
            "kT_d": nc.declare_dram_parameter("kT_d", [P, S], FP16, isOutput=True),
            "v_d": nc.declare_dram_parameter("v_d", [P, TT, HD + 1], BF16, isOutput=True),
        }
    with tile.TileContext(nc) as tc:
        with ExitStack() as ctx:
            _body(nc, tc, ctx, xT, wqkv, woT, tri, out_ext, taps)
    nc.finalize()
    return nc


def _body(nc, tc, ctx, xT, wqkv, woT, tri, out_ext, taps=None):
    # DRAM bounce buffers for the AllToAlls
    dram_pool = ctx.enter_context(tc.tile_pool(name="dram", bufs=1, space="DRAM"))
    a2a_in0 = dram_pool.tile([NCORES * JQ, TOK], BF16, name="a2a_in0", tag="ai0")
    a2a_out0 = dram_pool.tile([NCORES * JQ, TOK], BF16, name="a2a_out0", tag="ao0")
    a2a_in1 = [
        dram_pool.tile([NCORES * 2 * HD, TOK], BF16, name=f"a2a_in1{p}", tag=f"bi{p}")
        for p in range(2)
    ]
    a2a_out1 = [
        dram_pool.tile([NCORES * 2 * HD, TOK], BF16, name=f"a2a_out1{p}", tag=f"bo{p}")
        for p in range(2)
    ]
    singles = ctx.enter_context(tc.tile_pool(name="singles", bufs=1))
    xpool = ctx.enter_context(tc.tile_pool(name="xpool", bufs=2))
    vpool = ctx.enter_context(tc.tile_pool(name="vpool", bufs=2))
    pt_pool = ctx.enter_context(tc.tile_pool(name="pt", bufs=6))
    bc_sb = ctx.enter_context(tc.tile_pool(name="bc_sb", bufs=3))
    an_sb = ctx.enter_context(tc.tile_pool(name="an_sb", bufs=3))
    orhs = ctx.enter_context(tc.tile_pool(name="orhs", bufs=32))
    osb = ctx.enter_context(tc.tile_pool(name="osb", bufs=2))
    # PSUM: 8 banks of 2KB/partition total: 2 + 2*2 + 2 = 8
    ps_b = ctx.enter_context(tc.tile_pool(name="ps_b", bufs=2, space="PSUM"))
    ps_sc = ctx.enter_context(tc.tile_pool(name="ps_sc", bufs=2, space="PSUM"))
    ps_at = ctx.enter_context(tc.tile_pool(name="ps_at", bufs=2, space="PSUM"))

    # ---------------- persistent tiles ----------------
    wqkv_sb = singles.tile([P, DT, NQKV], FP16)
    _wsrc = wqkv.rearrange("(a p) n -> p a n", p=P)
    for c in range(DT):
        nc.scalar.dma_start(out=wqkv_sb[:, c:c + 1, :], in_=_wsrc[:, c:c + 1, :])
    woT_sb = singles.tile([P, DT, D], BF16)
    tri_sb = singles.tile([P, P], BF16)
    nc.scalar.dma_start(out=tri_sb, in_=tri[:, :])
    id_fp16 = singles.tile([P, P], FP16)
    make_identity(nc, id_fp16)
    qT_sb = singles.tile([P, 2, S], FP16)   # [64*hh+hd, pair, t]
    kT_sb = singles.tile([P, S], FP16)      # sm-scaled k, duplicated halves
    v_sb = singles.tile([P, TT, HD + 1], BF16)
    nc.vector.memset(v_sb, 1.0)             # col 64 stays 1.0 (sumexp trick)
    ones_sb = singles.tile([HD + 1, HD], BF16)
    nc.vector.memset(ones_sb, 1.0)

    def xb_load(tb):
        xb = xpool.tile([P, DT, BN], FP16, tag="xb")
        xsrc = xT[:, ts(tb, BN)].rearrange("(a p) m -> p a m", p=P)
        for c in range(4):
            nc.sync.dma_start(out=xb[:, 4 * c:4 * c + 4, :],
                              in_=xsrc[:, 4 * c:4 * c + 4, :])
        return xb

    def proj_ct(tb, xb, ct):
        """One 16-MM projection group (PE filler)."""
        ps = ps_b.tile([P, BN], F32, tag="mm")
        for d in range(DT):
            nc.tensor.matmul(
                ps, lhsT=wqkv_sb[:, d, ts(ct, P)], rhs=xb[:, d, :],
                start=(d == 0), stop=(d == DT - 1))
        if ct < 2:
            nc.vector.tensor_copy(qT_sb[:, ct, ts(tb, BN)], ps)
            return None
        # cols 256:320 = v feats (psum partitions 0:64),
        # cols 320:384 = k feats (partitions 64:128)
        nc.vector.tensor_copy(kT_sb[HD:P, ts(tb, BN)], ps[HD:P, :])
        nc.sync.dma_start(out=kT_sb[0:HD, ts(tb, BN)],
                          in_=kT_sb[HD:P, ts(tb, BN)])
        vtmp = vpool.tile([HD, BN], FP16, tag="vt")
        nc.vector.tensor_copy(vtmp, ps[0:HD, :])
        return vtmp

    def v_trans(tb, vtmp):
        """Transpose vT -> natural v (PE filler)."""
        for q4 in range(4):
            vt = ps_b.tile([P, HD], FP16, tag="mm")
            nc.tensor.transpose(vt, vtmp[:, ts(q4, P)], id_fp16[0:HD, 0:HD])
            nc.vector.tensor_copy(v_sb[:, 4 * tb + q4, 0:HD], vt)

    def proj_fillers(tb):
        xb = xb_load(tb)
        st = {}

        def f_ct(ct):
            def f():
                r = proj_ct(tb, xb, ct)
                if ct == 2:
                    st["vtmp"] = r
            return f
        return [f_ct(0), f_ct(1), f_ct(2), lambda: v_trans(tb, st["vtmp"])]

    def oproj_chunk(g, od, ork, korder=None):
        """16-MM o_proj group for output-dim block od (PE filler). korder
        lets group 1 accumulate even k-tiles (first A2A half) before odd."""
        ks = list(korder) if korder is not None else list(range(DT))
        pso = ps_b.tile([P, BN], F32, tag="mm", name=f"op{g}_{od}")
        for i, k in enumerate(ks):
            nc.tensor.matmul(
                pso, lhsT=ork[k], rhs=woT_sb[:, k, ds(od * BN, BN)],
                start=(i == 0), stop=(i == DT - 1))
        ot = osb.tile([P, BN], F32, tag="ot", name=f"ot{g}_{od}")
        nc.vector.tensor_copy(ot, pso)
        nc.sync.dma_start(out=out_ext[g, :, ds(od * BN, BN)], in_=ot)

    def trigger_a2a(src_t, dst_t):
        nc.gpsimd.collective_compute(
            "AllToAll", ALU.bypass,
            replica_groups=[list(range(NCORES))],
            ins=[src_t[:, :]],
            outs=[dst_t[:, :]])

    def att_block(b, fillers, pair_end=None):
        """Attention for query block b, software-pipelined, sprinkling
        `fillers` (independent PE work) between ap iterations."""
        na = 4 * (b + 1)
        g = b // 2
        fillers = list(fillers)
        stride = max(1, na // max(1, len(fillers)))
        slot = 0
        def do_norm(pair, atp):
            # normalize by sumexp (row 64), emit bf16 into the a2a input buffer
            for hh in range(2):
                se = bc_sb.tile([HD + 1, BN], F32, tag="se")
                nc.vector.tensor_copy(se[HD:HD + 1, :], atp[hh][HD:HD + 1, :])
                se0 = bc_sb.tile([1, BN], F32, tag="se0")
                nc.sync.dma_start(out=se0[0:1, :], in_=se[HD:HD + 1, :])
                rcp0 = bc_sb.tile([1, BN], F32, tag="rcp0")
                nc.vector.reciprocal_approx_fast(rcp0, se0)
                rcpb = bc_sb.tile([1, BN], BF16, tag="rcpb")
                nc.vector.tensor_copy(rcpb, rcp0)
                rbs = ps_b.tile([HD, BN], F32, tag="mm",
                                name=f"rbs{b}_{pair}_{hh}")
                nc.tensor.matmul(
                    rbs, lhsT=ones_sb[0:1, 0:HD],
                    rhs=rcpb[0:1, :], start=True, stop=True)
                rbs_sb = bc_sb.tile([HD, BN], F32, tag="rbs_sb")
                nc.vector.tensor_copy(rbs_sb, rbs)
                ans = an_sb.tile([HD, BN], BF16, tag="ans")
                nc.vector.tensor_mul(ans, atp[hh][0:HD, :], rbs_sb)
                for cch in range(4):
                    j = 4 * (b % 2) + cch
                    if g == 0:
                        dst = a2a_in0[ds(JQ * j + HD * (2 * pair + hh), HD), :]
                    else:
                        dst = a2a_in1[pair][ds(2 * HD * j + HD * hh, HD), :]
                    nc.sync.dma_start(out=dst, in_=ans[:, ts(cch, TOK)])
            if pair_end is not None:
                pair_end(pair)

        pending_norm = None
        for pair in range(2):
            atp = [ps_at.tile([HD + 1, BN], F32, tag="at",
                              name=f"at{b}_{pair}_{hh}") for hh in range(2)]
            pend = None
            for ap in range(na // 2):
                a0, a1 = 2 * ap, 2 * ap + 1
                off0 = max(0, a0 - 4 * b) * P
                off1 = max(0, a1 - 4 * b) * P
                len1 = BN - off1
                # 4 score MMs, hh-alternated so row groups 0/1 overlap
                scs = []
                for hh in range(2):
                    scs.append(ps_sc.tile([P, 2 * BN], F32, tag="sc",
                                          name=f"sc{b}_{pair}_{ap}_{hh}"))
                for hh in range(2):
                    rows = slice(HD * hh, HD * hh + HD)
                    nc.tensor.matmul(
                        scs[hh][:, off0:BN], lhsT=kT_sb[rows, ts(a0, P)],
                        rhs=qT_sb[rows, pair, ds(b * BN + off0, BN - off0)],
                        start=True, stop=True)
                for hh in range(2):
                    rows = slice(HD * hh, HD * hh + HD)
                    nc.tensor.matmul(
                        scs[hh][:, BN:BN + len1], lhsT=kT_sb[rows, ts(a1, P)],
                        rhs=qT_sb[rows, pair, ds(b * BN + off1, len1)],
                        start=True, stop=True)
                pts = []
                for hh in range(2):
                    pt = pt_pool.tile([P, 2 * BN], BF16, tag="pt",
                                      name=f"pt{b}_{pair}_{ap}_{hh}")
                    nc.scalar.activation(
                        out=pt[:, off0:BN + len1], in_=scs[hh][:, off0:BN + len1],
                        func=AF.Exp)
                    pts.append(pt)
                for hh in range(2):
                    if a0 >= 4 * b:
                        nc.vector.tensor_mul(
                            pts[hh][:, off0:off0 + P], pts[hh][:, off0:off0 + P],
                            tri_sb)
                    if a1 >= 4 * b:
                        nc.vector.tensor_mul(
                            pts[hh][:, BN:BN + P], pts[hh][:, BN:BN + P], tri_sb)
                # deferred previous-pair normalize: now the PE has this pair's
                # score MMs in front of it, so the K=1 broadcast matmul's wait
                # on the reciprocal chain is hidden
                if pending_norm is not None and ap == 1:
                    pending_norm()
                    pending_norm = None
                # AV matmuls of the previous ap (software pipelining)
                if pend is not None:
                    _issue_avs(atp, pend, na)
                pend = (a0, a1, off0, off1, len1, pts)
                slot += 1
                if fillers and slot % stride == 0:
                    fillers.pop(0)()
            _issue_avs(atp, pend, na)
            if pair == 0:
                pending_norm = (lambda atp=atp: do_norm(0, atp))
            else:
                do_norm(1, atp)
        for f in fillers:
            f()

    def _issue_avs(atp, pend, na):
        a0, a1, off0, off1, len1, pts = pend
        for hh in range(2):
            nc.tensor.matmul(
                atp[hh][:, off0:], lhsT=v_sb[:, a0, :],
                rhs=pts[hh][:, off0:BN], start=(a0 == 0), stop=False)
            nc.tensor.matmul(
                atp[hh][:, off1:], lhsT=v_sb[:, a1, :],
                rhs=pts[hh][:, BN:BN + len1], start=False, stop=(a1 == na - 1))

    # ---------------- schedule ----------------
    for f in proj_fillers(0):
        f()
    att_block(0, proj_fillers(1))
    # prefetch full WoT during the attention phase (gpsimd DMA queue; the
    # only other thing on gpsimd is the collective triggers, later)
    for c in range(DT):
        nc.gpsimd.dma_start(out=woT_sb[:, c:c + 1, :],
                            in_=woT.rearrange("(a p) n -> p a n", p=P)[:, c:c + 1, :])
    att_block(1, proj_fillers(2))
    trigger_a2a(a2a_in0, a2a_out0)
    att_block(2, proj_fillers(3))
    ork0 = []
    for k in range(DT):
        rt = orhs.tile([P, TOK], BF16, tag="rt", name=f"rt0_{k}")
        nc.sync.dma_start(out=rt, in_=a2a_out0[ts(k, P), :])
        ork0.append(rt)
    # ork1: even k-tiles come from the pair-0 half A2A, odd from pair-1
    ork1 = [None] * DT

    def b3_pair_end(pair):
        trigger_a2a(a2a_in1[pair], a2a_out1[pair])
        for s in range(NCORES):
            rt = orhs.tile([P, TOK], BF16, tag="rt", name=f"rt1_{2 * s + pair}")
            nc.sync.dma_start(out=rt, in_=a2a_out1[pair][ts(s, P), :])
            ork1[2 * s + pair] = rt

    att_block(3, [lambda: oproj_chunk(0, 0, ork0), lambda: oproj_chunk(0, 1, ork0)],
              pair_end=b3_pair_end)
    # the deferred o_proj-0 chunks keep the PE busy during A2A#1b
    oproj_chunk(0, 2, ork0)
    oproj_chunk(0, 3, ork0)
    if taps is not None:
        nc.sync.dma_start(out=taps["qT_d"][:, :, :], in_=qT_sb)
        nc.sync.dma_start(out=taps["kT_d"][:, :], in_=kT_sb)
        nc.sync.dma_start(out=taps["v_d"][:, :, :], in_=v_sb)
    evens_then_odds = [0, 2, 4, 6, 8, 10, 12, 14, 1, 3, 5, 7, 9, 11, 13, 15]
    for od in range(4):
        oproj_chunk(1, od, ork1, korder=evens_then_odds)


# ---------------- host side ----------------

def prep_in_maps(x, Wq, Wk, Wv, Wo):
    bf = ml_dtypes.bfloat16
    xTh = np.ascontiguousarray(x.reshape(S, D).T.astype(np.float16))
    tri_h = np.ascontiguousarray(
        (np.arange(P)[:, None] <= np.arange(P)[None, :]).astype(bf))
    woT_h = np.ascontiguousarray(Wo.T.astype(bf))
    in_maps = []
    for c in range(NCORES):
        wq = Wq[c * JQ:(c + 1) * JQ, :].T
        wk = Wk[c * HD:(c + 1) * HD, :].T * SM
        wv = Wv[c * HD:(c + 1) * HD, :].T
        wqkv_h = np.ascontiguousarray(
            np.concatenate([wq, wv, wk], axis=1).astype(np.float16))
        in_maps.append({"xT": xTh, "wqkv": wqkv_h, "woT": woT_h, "tri": tri_h})
    return in_maps


def unshard(results):
    out = np.empty((S, D), dtype=np.float32)
    for c in range(NCORES):
        o = np.asarray(results[c]["out"]).reshape(2, TOK, D)
        out[c * TOK:(c + 1) * TOK, :] = o[0]
        out[S // 2 + c * TOK:S // 2 + (c + 1) * TOK, :] = o[1]
    return out.reshape(1, S, D)


def kernel(x, Wq, Wk, Wv, Wo):
    from concourse.bass_utils import run_bass_kernel_spmd
    nc = build_nc()
    in_maps = prep_in_maps(x, Wq, Wk, Wv, Wo)
    res = run_bass_kernel_spmd(nc, in_maps, core_ids=list(range(NCORES)))
    return unshard(res.results)


# revision 26
# speedup vs baseline: 1.0417x; 1.0417x over previous
"""Trainium2 Bass kernel for GQA attention (8-core SPMD, tensor-parallel heads).

Per-core shard c of 8 (4 q heads, 1 kv head):
  Projection (weights stationary, fp16): qT/kT/vT computed directly TRANSPOSED:
    psum[feat, tok] = WqkvT[d, feat].T @ xT[d, tok]. sm folded into Wk on host.
    No int8-quantization emulation (the reference's int8 round-trip is ~1%
    noise on the output; tolerance is 2e-2). v is PE-transposed back to
    natural [tok, hd] layout for the AV matmul.
  Attention: scoresT[t2, t1] = kT.T @ qT, two heads row-tiled concurrently
    (K=64 each, distinct row groups); two key-tiles packed gaplessly into one
    [128, 1024] 2-bank psum tile so each exp ACTIVATE covers ~1024 columns.
    p = exp(scoresT) -> bf16, causal tri-mask on diagonal tiles,
    attT[hd, t1] = v_aug.T @ p with ones column -> row 64 = sumexp; normalize
    via reciprocal + K=1-matmul partition broadcast (gpsimd must stay nearly
    empty: a collective trigger blocks its engine until the CC stream is past
    the variable-length entry barrier).
  Schedule: the attention inner loop is software-pipelined (ap's score MMs
    issue before ap-1's AV MMs) and projection ct-groups / o_proj chunks are
    interleaved as independent PE filler between ap iterations.
  o_proj (token-sharded): AllToAlls redistribute att [256 feat, tokens] ->
    [2048 feat, 128-token chunk per core]; each core holds FULL WoT and
    computes out[tok_chunk, :] = att_chunk.T @ Wo.T. Host stitches tokens.
    The second token group's A2A is split by head-pair so the first half
    flies during b3 pair 1 and only a 256KB collective plus the odd k-tiles
    of o_proj remain in the tail.
"""

import numpy as np
import ml_dtypes
from contextlib import ExitStack

import concourse.bass as bass
import concourse.mybir as mybir
import concourse.tile as tile
from concourse import bacc
from concourse.bass import ts, ds
from concourse.masks import make_identity

NCORES = 8
P = 128
S = 2048          # tokens
D = 2048          # model dim
HD = 64           # head dim
NHL = 4           # q heads per core
JQ = NHL * HD     # 256 (q feature rows per core)
NQKV = JQ + 2 * HD  # 384 wqkv columns per core (q0..q3, v, k)
TT = S // P       # 16 token tiles
DT = D // P       # 16 d tiles
NB = 4            # t1 blocks
BN = S // NB      # 512
TOK = 128         # a2a per-core token chunk
SM = HD ** -0.5   # 0.125 (folded into Wk on host)
F32 = mybir.dt.float32
BF16 = mybir.dt.bfloat16
FP16 = mybir.dt.float16
AF = mybir.ActivationFunctionType
ALU = mybir.AluOpType


def build_nc(debug_taps=False):
    nc = bacc.Bacc(target_bir_lowering=False, debug=False, num_devices=NCORES)
    xT = nc.declare_dram_parameter("xT", [D, S], FP16, isOutput=False)
    wqkv = nc.declare_dram_parameter("wqkv", [D, NQKV], FP16, isOutput=False)
    woT = nc.declare_dram_parameter("woT", [D, D], BF16, isOutput=False)
    tri = nc.declare_dram_parameter("tri", [P, P], BF16, isOutput=False)
    out_ext = nc.declare_dram_parameter("out", [2, P, D], F32, isOutput=True)

    taps = None
    if debug_taps:
        taps = {
            "qT_d": nc.declare_dram_parameter("qT_d", [P, 2, S], FP16, isOutput=True),
            "kT_d": nc.declare_dram_parameter("kT_d", [P, S], FP16, isOutput=True),
            "v_d": nc.declare_dram_parameter("v_d", [P, TT, HD + 1], BF16, isOutput=True),
        }
    with tile.TileContext(nc) as tc:
        with ExitStack() as ctx:
            _body(nc, tc, ctx, xT, wqkv, woT, tri, out_ext, taps)
    nc.finalize()
    return nc


def _body(nc, tc, ctx, xT, wqkv, woT, tri, out_ext, taps=None):
    # DRAM bounce buffers for the AllToAlls
    dram_pool = ctx.enter_context(tc.tile_pool(name="dram", bufs=1, space="DRAM"))
    a2a_in0 = dram_pool.tile([NCORES * JQ, TOK], BF16, name="a2a_in0", tag="ai0")
    a2a_out0 = dram_pool.tile([NCORES * JQ, TOK], BF16, name="a2a_out0", tag="ao0")
    a2a_in1 = [
        dram_pool.tile([NCORES * 2 * HD, TOK], BF16, name=f"a2a_in1{p}", tag=f"bi{p}")
        for p in range(2)
    ]
    a2a_out1 = [
        dram_pool.tile([NCORES * 2 * HD, TOK], BF16, name=f"a2a_out1{p}", tag=f"bo{p}")
        for p in range(2)
    ]
    singles = ctx.enter_context(tc.tile_pool(name="singles", bufs=1))
    xpool = ctx.enter_context(tc.tile_pool(name="xpool", bufs=2))
    vpool = ctx.enter_context(tc.tile_pool(name="vpool", bufs=2))
    pt_pool = ctx.enter_context(tc.tile_pool(name="pt", bufs=6))
    bc_sb = ctx.enter_context(tc.tile_pool(name="bc_sb", bufs=4))
    an_sb = ctx.enter_context(tc.tile_pool(name="an_sb", bufs=4))
    orhs = ctx.enter_context(tc.tile_pool(name="orhs", bufs=32))
    osb = ctx.enter_context(tc.tile_pool(name="osb", bufs=2))
    # PSUM: 8 banks of 2KB/partition total: 2 + 2*2 + 2 = 8
    ps_b = ctx.enter_context(tc.tile_pool(name="ps_b", bufs=2, space="PSUM"))
    ps_sc = ctx.enter_context(tc.tile_pool(name="ps_sc", bufs=2, space="PSUM"))
    ps_at = ctx.enter_context(tc.tile_pool(name="ps_at", bufs=2, space="PSUM"))

    # ---------------- persistent tiles ----------------
    wqkv_sb = singles.tile([P, DT, NQKV], FP16)
    _wsrc = wqkv.rearrange("(a p) n -> p a n", p=P)
    for c in range(DT):
        nc.scalar.dma_start(out=wqkv_sb[:, c:c + 1, :], in_=_wsrc[:, c:c + 1, :])
    woT_sb = singles.tile([P, DT, D], BF16)
    tri_sb = singles.tile([P, P], BF16)
    nc.scalar.dma_start(out=tri_sb, in_=tri[:, :])
    id_fp16 = singles.tile([P, P], FP16)
    make_identity(nc, id_fp16)
    qT_sb = singles.tile([P, 2, S], FP16)   # [64*hh+hd, pair, t]
    kT_sb = singles.tile([P, S], FP16)      # sm-scaled k, duplicated halves
    v_sb = singles.tile([P, TT, HD + 1], BF16)
    nc.vector.memset(v_sb, 1.0)             # col 64 stays 1.0 (sumexp trick)
    ones_sb = singles.tile([HD + 1, HD], BF16)
    nc.vector.memset(ones_sb, 1.0)

    def xb_load(tb):
        xb = xpool.tile([P, DT, BN], FP16, tag="xb")
        xsrc = xT[:, ts(tb, BN)].rearrange("(a p) m -> p a m", p=P)
        for c in range(8):
            nc.sync.dma_start(out=xb[:, 2 * c:2 * c + 2, :],
                              in_=xsrc[:, 2 * c:2 * c + 2, :])
        return xb

    def proj_ct(tb, xb, ct):
        """One 16-MM projection group (PE filler)."""
        ps = ps_b.tile([P, BN], F32, tag="mm")
        for d in range(DT):
            nc.tensor.matmul(
                ps, lhsT=wqkv_sb[:, d, ts(ct, P)], rhs=xb[:, d, :],
                start=(d == 0), stop=(d == DT - 1))
        if ct < 2:
            nc.vector.tensor_copy(qT_sb[:, ct, ts(tb, BN)], ps)
            return None
        # cols 256:320 = v feats (psum partitions 0:64),
        # cols 320:384 = k feats (partitions 64:128)
        nc.vector.tensor_copy(kT_sb[HD:P, ts(tb, BN)], ps[HD:P, :])
        nc.sync.dma_start(out=kT_sb[0:HD, ts(tb, BN)],
                          in_=kT_sb[HD:P, ts(tb, BN)])
        vtmp = vpool.tile([HD, BN], FP16, tag="vt")
        nc.vector.tensor_copy(vtmp, ps[0:HD, :])
        return vtmp

    def v_trans(tb, vtmp):
        """Transpose vT -> natural v (PE filler)."""
        for q4 in range(4):
            vt = ps_b.tile([P, HD], FP16, tag="mm")
            nc.tensor.transpose(vt, vtmp[:, ts(q4, P)], id_fp16[0:HD, 0:HD])
            nc.vector.tensor_copy(v_sb[:, 4 * tb + q4, 0:HD], vt)

    def proj_fillers(tb):
        xb = xb_load(tb)
        st = {}

        def f_ct(ct):
            def f():
                r = proj_ct(tb, xb, ct)
                if ct == 2:
                    st["vtmp"] = r
            return f
        return [f_ct(0), f_ct(1), f_ct(2), lambda: v_trans(tb, st["vtmp"])]

    def oproj_chunk(g, od, ork, korder=None, pso=None, finish=True):
        """o_proj accumulation group for output-dim block od (PE filler).
        korder restricts/reorders the k-tiles; pass pso + finish=False to
        accumulate in two phases (even k-tiles before the odd half-A2A lands)."""
        ks = list(korder) if korder is not None else list(range(DT))
        if pso is None:
            pso = ps_b.tile([P, BN], F32, tag="mm", name=f"op{g}_{od}")
        first = not finish or len(ks) == DT
        for i, k in enumerate(ks):
            nc.tensor.matmul(
                pso, lhsT=ork[k], rhs=woT_sb[:, k, ds(od * BN, BN)],
                start=(first and i == 0), stop=(finish and i == len(ks) - 1))
        if not finish:
            return pso
        ot = osb.tile([P, BN], F32, tag="ot", name=f"ot{g}_{od}")
        nc.vector.tensor_copy(ot, pso)
        nc.sync.dma_start(out=out_ext[g, :, ds(od * BN, BN)], in_=ot)

    def trigger_a2a(src_t, dst_t):
        nc.gpsimd.collective_compute(
            "AllToAll", ALU.bypass,
            replica_groups=[list(range(NCORES))],
            ins=[src_t[:, :]],
            outs=[dst_t[:, :]])

    def att_block(b, fillers, pair_end=None):
        """Attention for query block b, software-pipelined, sprinkling
        `fillers` (independent PE work) between ap iterations."""
        na = 4 * (b + 1)
        g = b // 2
        fillers = list(fillers)
        stride = max(1, na // max(1, len(fillers)))
        slot = 0
        def do_norm(pair, atp):
            # normalize by sumexp (row 64), emit bf16 into the a2a input buffer
            for hh in range(2):
                se = bc_sb.tile([HD + 1, BN], F32, tag="se")
                nc.vector.tensor_copy(se[HD:HD + 1, :], atp[hh][HD:HD + 1, :])
                se0 = bc_sb.tile([1, BN], F32, tag="se0")
                nc.sync.dma_start(out=se0[0:1, :], in_=se[HD:HD + 1, :])
                rcp0 = bc_sb.tile([1, BN], F32, tag="rcp0")
                nc.vector.reciprocal_approx_fast(rcp0, se0)
                rcpb = bc_sb.tile([1, BN], BF16, tag="rcpb")
                nc.vector.tensor_copy(rcpb, rcp0)
                rbs = ps_b.tile([HD, BN], F32, tag="mm",
                                name=f"rbs{b}_{pair}_{hh}")
                nc.tensor.matmul(
                    rbs, lhsT=ones_sb[0:1, 0:HD],
                    rhs=rcpb[0:1, :], start=True, stop=True)
                rbs_sb = bc_sb.tile([HD, BN], F32, tag="rbs_sb")
                nc.vector.tensor_copy(rbs_sb, rbs)
                ans = an_sb.tile([HD, BN], BF16, tag="ans")
                nc.vector.tensor_mul(ans, atp[hh][0:HD, :], rbs_sb)
                for cch in range(4):
                    j = 4 * (b % 2) + cch
                    if g == 0:
                        dst = a2a_in0[ds(JQ * j + HD * (2 * pair + hh), HD), :]
                    else:
                        dst = a2a_in1[pair][ds(2 * HD * j + HD * hh, HD), :]
                    nc.sync.dma_start(out=dst, in_=ans[:, ts(cch, TOK)])
            if pair_end is not None:
                pair_end(pair)

        pending_norm = None
        for pair in range(2):
            atp = [ps_at.tile([HD + 1, BN], F32, tag="at",
                              name=f"at{b}_{pair}_{hh}") for hh in range(2)]
            pend = None
            for ap in range(na // 2):
                a0, a1 = 2 * ap, 2 * ap + 1
                off0 = max(0, a0 - 4 * b) * P
                off1 = max(0, a1 - 4 * b) * P
                len1 = BN - off1
                # 4 score MMs, hh-alternated so row groups 0/1 overlap
                scs = []
                for hh in range(2):
                    scs.append(ps_sc.tile([P, 2 * BN], F32, tag="sc",
                                          name=f"sc{b}_{pair}_{ap}_{hh}"))
                for hh in range(2):
                    rows = slice(HD * hh, HD * hh + HD)
                    nc.tensor.matmul(
                        scs[hh][:, off0:BN], lhsT=kT_sb[rows, ts(a0, P)],
                        rhs=qT_sb[rows, pair, ds(b * BN + off0, BN - off0)],
                        start=True, stop=True)
                for hh in range(2):
                    rows = slice(HD * hh, HD * hh + HD)
                    nc.tensor.matmul(
                        scs[hh][:, BN:BN + len1], lhsT=kT_sb[rows, ts(a1, P)],
                        rhs=qT_sb[rows, pair, ds(b * BN + off1, len1)],
                        start=True, stop=True)
                pts = []
                for hh in range(2):
                    pt = pt_pool.tile([P, 2 * BN], BF16, tag="pt",
                                      name=f"pt{b}_{pair}_{ap}_{hh}")
                    nc.scalar.activation(
                        out=pt[:, off0:BN + len1], in_=scs[hh][:, off0:BN + len1],
                        func=AF.Exp)
                    pts.append(pt)
                for hh in range(2):
                    if a0 >= 4 * b:
                        nc.vector.tensor_mul(
                            pts[hh][:, off0:off0 + P], pts[hh][:, off0:off0 + P],
                            tri_sb)
                    if a1 >= 4 * b:
                        nc.vector.tensor_mul(
                            pts[hh][:, BN:BN + P], pts[hh][:, BN:BN + P], tri_sb)
                # deferred previous-pair normalize: now the PE has this pair's
                # score MMs in front of it, so the K=1 broadcast matmul's wait
                # on the reciprocal chain is hidden
                if pending_norm is not None and ap == 1:
                    pending_norm()
                    pending_norm = None
                # AV matmuls of the previous ap (software pipelining)
                if pend is not None:
                    _issue_avs(atp, pend, na)
                pend = (a0, a1, off0, off1, len1, pts)
                slot += 1
                if fillers and slot % stride == 0:
                    fillers.pop(0)()
            _issue_avs(atp, pend, na)
            if pair == 0:
                pending_norm = (lambda atp=atp: do_norm(0, atp))
            else:
                do_norm(1, atp)
        for f in fillers:
            f()

    def _issue_avs(atp, pend, na):
        a0, a1, off0, off1, len1, pts = pend
        for hh in range(2):
            nc.tensor.matmul(
                atp[hh][:, off0:], lhsT=v_sb[:, a0, :],
                rhs=pts[hh][:, off0:BN], start=(a0 == 0), stop=False)
            nc.tensor.matmul(
                atp[hh][:, off1:], lhsT=v_sb[:, a1, :],
                rhs=pts[hh][:, BN:BN + len1], start=False, stop=(a1 == na - 1))

    # ---------------- schedule ----------------
    for f in proj_fillers(0):
        f()
    att_block(0, proj_fillers(1))
    # prefetch full WoT during the attention phase (gpsimd DMA queue; the
    # only other thing on gpsimd is the collective triggers, later)
    for c in range(DT):
        nc.gpsimd.dma_start(out=woT_sb[:, c:c + 1, :],
                            in_=woT.rearrange("(a p) n -> p a n", p=P)[:, c:c + 1, :])
    att_block(1, proj_fillers(2))
    trigger_a2a(a2a_in0, a2a_out0)
    att_block(2, proj_fillers(3))
    ork0 = []
    for k in range(DT):
        rt = orhs.tile([P, TOK], BF16, tag="rt", name=f"rt0_{k}")
        nc.sync.dma_start(out=rt, in_=a2a_out0[ts(k, P), :])
        ork0.append(rt)
    # ork1: even k-tiles come from the pair-0 half A2A, odd from pair-1
    ork1 = [None] * DT

    def b3_pair_end(pair):
        trigger_a2a(a2a_in1[pair], a2a_out1[pair])
        for s in range(NCORES):
            rt = orhs.tile([P, TOK], BF16, tag="rt", name=f"rt1_{2 * s + pair}")
            nc.sync.dma_start(out=rt, in_=a2a_out1[pair][ts(s, P), :])
            ork1[2 * s + pair] = rt

    att_block(3, [lambda: oproj_chunk(0, 0, ork0), lambda: oproj_chunk(0, 1, ork0)],
              pair_end=b3_pair_end)
    # tail: the deferred o_proj-0 chunks plus o_proj-1's even k-tiles (whose
    # data arrived with A2A#1a) keep the PE busy while A2A#1b is in flight;
    # only the odd-k accumulations actually wait on it. PSUM note: ps_b has 2
    # buffers, so at most two o_proj-1 psum groups are left open at a time.
    oproj_chunk(0, 2, ork0)
    oproj_chunk(0, 3, ork0)
    if taps is not None:
        nc.sync.dma_start(out=taps["qT_d"][:, :, :], in_=qT_sb)
        nc.sync.dma_start(out=taps["kT_d"][:, :], in_=kT_sb)
        nc.sync.dma_start(out=taps["v_d"][:, :, :], in_=v_sb)
    evens = [0, 2, 4, 6, 8, 10, 12, 14]
    odds = [1, 3, 5, 7, 9, 11, 13, 15]
    pso0 = oproj_chunk(1, 0, ork1, korder=evens, finish=False)
    pso1 = oproj_chunk(1, 1, ork1, korder=evens, finish=False)
    oproj_chunk(1, 0, ork1, korder=odds, pso=pso0)
    pso2 = oproj_chunk(1, 2, ork1, korder=evens, finish=False)
    oproj_chunk(1, 1, ork1, korder=odds, pso=pso1)
    pso3 = oproj_chunk(1, 3, ork1, korder=evens, finish=False)
    oproj_chunk(1, 2, ork1, korder=odds, pso=pso2)
    oproj_chunk(1, 3, ork1, korder=odds, pso=pso3)


# ---------------- host side ----------------

def prep_in_maps(x, Wq, Wk, Wv, Wo):
    bf = ml_dtypes.bfloat16
    xTh = np.ascontiguousarray(x.reshape(S, D).T.astype(np.float16))
    tri_h = np.ascontiguousarray(
        (np.arange(P)[:, None] <= np.arange(P)[None, :]).astype(bf))
    woT_h = np.ascontiguousarray(Wo.T.astype(bf))
    in_maps = []
    for c in range(NCORES):
        wq = Wq[c * JQ:(c + 1) * JQ, :].T
        wk = Wk[c * HD:(c + 1) * HD, :].T * SM
        wv = Wv[c * HD:(c + 1) * HD, :].T
        wqkv_h = np.ascontiguousarray(
            np.concatenate([wq, wv, wk], axis=1).astype(np.float16))
        in_maps.append({"xT": xTh, "wqkv": wqkv_h, "woT": woT_h, "tri": tri_h})
    return in_maps


def unshard(results):
    out = np.empty((S, D), dtype=np.float32)
    for c in range(NCORES):
        o = np.asarray(results[c]["out"]).reshape(2, TOK, D)
        out[c * TOK:(c + 1) * TOK, :] = o[0]
        out[S // 2 + c * TOK:S // 2 + (c + 1) * TOK, :] = o[1]
    return out.reshape(1, S, D)


def kernel(x, Wq, Wk, Wv, Wo):
    from concourse.bass_utils import run_bass_kernel_spmd
    nc = build_nc()
    in_maps = prep_in_maps(x, Wq, Wk, Wv, Wo)
    res = run_bass_kernel_spmd(nc, in_maps, core_ids=list(range(NCORES)))
    return unshard(res.results)
